# revision 4
# baseline (speedup 1.0000x reference)
"""Multi-head graph attention (GAT) Bass kernel for Trainium2, 8 NeuronCores.

Shards the destination-node (i) dimension across 8 cores. Each core:
  - computes Wh = h @ W (replicated), attention coefficients f_j (all nodes)
    and f_i (its 512-row slice),
  - computes P^T[j, i] = exp(leaky_relu(f_i[i] + f_j[j], 0.2)) tile by tile
    (j on partitions) via two ScalarE passes (Prelu with per-partition bias,
    then Exp emitting float32r),
  - aggregates out^T[d, i] = sum_j Wh[j, d] * P^T[j, i] on the PE in float32r
    with an appended ones column that yields the softmax denominator S for
    free,
  - PE-transposes P^T chunks back to i-major, evacuates them unscaled into
    contiguous per-head staging buffers, scales in place by 1/S and DMAs
    per-head attention slices out.
The host stacks the per-head planes and concatenates the per-core row slices.
"""

import numpy as np

N = 4096
D = 256
H = 4
OD = 64
NCORES = 8
IS = N // NCORES  # 512 rows of i per core
P = 128
NJC = N // P  # 32 j-chunks
NIC = IS // P  # 4 i-chunks per core

_cache = {}


def _build():
    from contextlib import ExitStack

    import concourse.bacc as bacc
    import concourse.mybir as mybir
    import concourse.tile as tile
    from concourse import masks

    dt = mybir.dt
    AF = mybir.ActivationFunctionType

    nc = bacc.Bacc("TRN2", target_bir_lowering=False, debug=False)

    h_d = nc.dram_tensor("h", [N, D], dt.float32, kind="ExternalInput").ap()
    hi_d = nc.dram_tensor("hi", [IS, D], dt.float32, kind="ExternalInput").ap()
    w_d = nc.dram_tensor("W", [D, D], dt.float32, kind="ExternalInput").ap()
    a_d = nc.dram_tensor("a", [H, 2 * OD], dt.float32, kind="ExternalInput").ap()

    attn_d = [
        nc.dram_tensor(f"attn{hh}", [IS, N], dt.float32, kind="ExternalOutput").ap()
        for hh in range(H)
    ]
    outp_d = nc.dram_tensor("outp", [IS, D], dt.float32, kind="ExternalOutput").ap()

    with tile.TileContext(nc) as tc, ExitStack() as ctx:
        # ---------------- persistent small tensors ----------------
        const_pool = ctx.enter_context(tc.tile_pool(name="const", bufs=1))
        ident_f = const_pool.tile([P, P], dt.float32)
        masks.make_identity(nc, ident_f[:])
        ident_r = const_pool.tile([P, P], dt.float32r)
        nc.vector.tensor_copy(ident_r[:], ident_f[:])
        ones_f = const_pool.tile([P, 1], dt.float32)
        nc.vector.memset(ones_f[:], 1.0)

        # AW[p, jc, h, 0:64] = Wh[jc*128+p, h*64:(h+1)*64]; AW[..., 64] = 1.0
        aw = const_pool.tile([P, NJC, H, OD + 1], dt.float32r)
        fj = const_pool.tile([P, NJC, H], dt.float32)
        fib = const_pool.tile([P, H, IS], dt.float32)
        sinv = const_pool.tile([P, NIC, H], dt.float32)
        out_sb = const_pool.tile([P, NIC, D], dt.float32)

        # ---------------- preamble (freed before the main loop) -------------
        with tc.tile_pool(name="pre", bufs=1) as pre, tc.tile_pool(
            name="pre_ps", bufs=2, space="PSUM"
        ) as pre_ps:
            h_sb = pre.tile([P, NJC, D], dt.float32)
            nc.sync.dma_start(h_sb[:], h_d.rearrange("(k p) d -> p k d", p=P))
            hi_sb = pre.tile([P, NIC, D], dt.float32)
            nc.sync.dma_start(hi_sb[:], hi_d.rearrange("(k p) d -> p k d", p=P))
            w_sb = pre.tile([P, 2, D], dt.float32)
            nc.sync.dma_start(w_sb[:], w_d.rearrange("(k p) c -> p k c", p=P))

            # block-diagonal attention vectors: A_*_BD[64*(hh%2):, hh//2, hh]
            a_dst = pre.tile([P, 2, H], dt.float32)
            a_src = pre.tile([P, 2, H], dt.float32)
            nc.vector.memset(a_dst[:], 0.0)
            nc.vector.memset(a_src[:], 0.0)
            for hh in range(H):
                ps = (hh % 2) * OD
                dc = hh // 2
                nc.sync.dma_start(
                    a_dst[ps : ps + OD, dc, hh : hh + 1],
                    a_d[hh : hh + 1, 0:OD].rearrange("o d -> d o"),
                )
                nc.sync.dma_start(
                    a_src[ps : ps + OD, dc, hh : hh + 1],
                    a_d[hh : hh + 1, OD : 2 * OD].rearrange("o d -> d o"),
                )

            # hT[p, dc, n] = h[n, dc*128+p]
            h_t = pre.tile([P, 2, N], dt.float32)
            for kt in range(NJC):
                for dc in range(2):
                    tp = pre_ps.tile([P, P], dt.float32, tag="pre_tp")
                    nc.tensor.transpose(
                        tp[:], h_sb[:, kt, dc * P : (dc + 1) * P], ident_f[:]
                    )
                    nc.vector.tensor_copy(h_t[:, dc, kt * P : (kt + 1) * P], tp[:])
            hi_t = pre.tile([P, 2, IS], dt.float32)
            for kt in range(NIC):
                for dc in range(2):
                    tp = pre_ps.tile([P, P], dt.float32, tag="pre_tp")
                    nc.tensor.transpose(
                        tp[:], hi_sb[:, kt, dc * P : (dc + 1) * P], ident_f[:]
                    )
                    nc.vector.tensor_copy(hi_t[:, dc, kt * P : (kt + 1) * P], tp[:])

            # WhT[p, cc, n] = Wh[n, cc*128+p]
            wh_t = pre.tile([P, 2, N], dt.float32)
            for cc in range(2):
                for ng in range(8):
                    ps_w = pre_ps.tile([P, 512], dt.float32, tag="pre_mm")
                    for dc in range(2):
                        nc.tensor.matmul(
                            ps_w[:],
                            w_sb[:, dc, cc * P : (cc + 1) * P],
                            h_t[:, dc, ng * 512 : (ng + 1) * 512],
                            start=(dc == 0),
                            stop=(dc == 1),
                        )
                    nc.vector.tensor_copy(wh_t[:, cc, ng * 512 : (ng + 1) * 512], ps_w[:])
            whi_t = pre.tile([P, 2, IS], dt.float32)
            for cc in range(2):
                ps_w = pre_ps.tile([P, 512], dt.float32, tag="pre_mm")
                for dc in range(2):
                    nc.tensor.matmul(
                        ps_w[:],
                        w_sb[:, dc, cc * P : (cc + 1) * P],
                        hi_t[:, dc, :],
                        start=(dc == 0),
                        stop=(dc == 1),
                    )
                nc.vector.tensor_copy(whi_t[:, cc, :], ps_w[:])

            # AW from WhT via PE transposes (round to f32r on evac)
            for cc in range(2):
                for jc in range(NJC):
                    tp = pre_ps.tile([P, P], dt.float32, tag="pre_tp")
                    nc.tensor.transpose(
                        tp[:], wh_t[:, cc, jc * P : (jc + 1) * P], ident_f[:]
                    )
                    nc.vector.tensor_copy(
                        aw[:, jc, 2 * cc : 2 * cc + 2, 0:OD], tp[:]
                    )
            nc.vector.tensor_copy(
                aw[:, :, :, OD : OD + 1],
                ones_f[:].to_broadcast([P, NJC, H, 1]),
            )

            # f_j columns
            for jc in range(NJC):
                ps_f = pre_ps.tile([P, H], dt.float32, tag="pre_f")
                for dc in range(2):
                    nc.tensor.matmul(
                        ps_f[:],
                        wh_t[:, dc, jc * P : (jc + 1) * P],
                        a_src[:, dc, :],
                        start=(dc == 0),
                        stop=(dc == 1),
                    )
                nc.vector.tensor_copy(fj[:, jc, :], ps_f[:])

            # f_i rows -> broadcast tiles
            for hh in range(H):
                ps_f = pre_ps.tile([1, IS], dt.float32, tag="pre_fi")
                for dc in range(2):
                    nc.tensor.matmul(
                        ps_f[:],
                        a_dst[:, dc, hh : hh + 1],
                        whi_t[:, dc, :],
                        start=(dc == 0),
                        stop=(dc == 1),
                    )
                fi_row = pre.tile([1, IS], dt.float32, tag="fi_row")
                nc.vector.tensor_copy(fi_row[:], ps_f[:])
                nc.gpsimd.partition_broadcast(fib[:, hh, :], fi_row[:])

        # ---------------- main loop ----------------
        et_pool = ctx.enter_context(tc.tile_pool(name="et", bufs=2))
        pt_pool = ctx.enter_context(tc.tile_pool(name="pt", bufs=3))
        stage_pool = ctx.enter_context(tc.tile_pool(name="stage", bufs=5))
        agg_ps_pool = ctx.enter_context(tc.tile_pool(name="agg_ps", bufs=2, space="PSUM"))
        tp_ps_pool = ctx.enter_context(tc.tile_pool(name="tp_ps", bufs=3, space="PSUM"))
        sm_ps_pool = ctx.enter_context(tc.tile_pool(name="sm_ps", bufs=1, space="PSUM"))
        sm_sb_pool = ctx.enter_context(tc.tile_pool(name="sm_sb", bufs=2))

        for hh in range(H):
            agg_ps = agg_ps_pool.tile([OD + 1, 512], dt.float32)
            stage = [
                stage_pool.tile([P, N], dt.float32, name=f"stage{c}", tag="st") for c in range(NIC)
            ]
            for jg in range(NJC // 4):
                et = et_pool.tile([P, 4, 512], dt.float32)
                for u in range(4):
                    jc = jg * 4 + u
                    nc.scalar.activation(
                        et[:, u, :],
                        fib[:, hh, :],
                        AF.Prelu,
                        bias=fj[:, jc, hh : hh + 1],
                        scale=1.0,
                        alpha=0.2,
                    )
                pt = pt_pool.tile([P, 4, 512], dt.float32r)
                nc.scalar.activation(pt[:], et[:], AF.Exp)
                for u in range(4):
                    jc = jg * 4 + u
                    nc.tensor.matmul(
                        agg_ps[:],
                        aw[:, jc, hh, :],
                        pt[:, u, :],
                        start=(jc == 0),
                        stop=(jc == NJC - 1),
                    )
                for c in range(NIC):
                    ps_t = tp_ps_pool.tile([P, 4, P], dt.float32r)
                    for u in range(4):
                        nc.tensor.transpose(
                            ps_t[:, u, :], pt[:, u, c * P : (c + 1) * P], ident_r[:]
                        )
                    nc.vector.tensor_copy(
                        stage[c][:, jg * 512 : (jg + 1) * 512], ps_t[:]
                    )

            # softmax denominators -> sinv columns
            s_row = sm_sb_pool.tile([1, 512], dt.float32, tag="s_row")
            nc.vector.tensor_copy(s_row[:], agg_ps[OD : OD + 1, :])
            for c in range(NIC):
                ps_s = sm_ps_pool.tile([P, 1], dt.float32, tag="ps_s")
                nc.tensor.transpose(
                    ps_s[:], s_row[0:1, c * P : (c + 1) * P], ident_f[0:1, 0:1]
                )
                nc.vector.reciprocal(sinv[:, c, hh : hh + 1], ps_s[:])

            # scale attention rows in place and store
            for c in range(NIC):
                nc.vector.tensor_scalar_mul(
                    stage[c][:], stage[c][:], sinv[:, c, hh : hh + 1]
                )
                nc.sync.dma_start(attn_d[hh][c * P : (c + 1) * P, :], stage[c][:])

            # aggregated output: transpose [65,512] psum back to i-major
            o_sb = sm_sb_pool.tile([OD + 1, 512], dt.float32, tag="o_sb")
            nc.vector.tensor_copy(o_sb[:], agg_ps[:])
            for c in range(NIC):
                ps_o = sm_ps_pool.tile([P, OD], dt.float32, tag="ps_o")
                nc.tensor.transpose(
                    ps_o[:], o_sb[0:OD, c * P : (c + 1) * P], ident_f[0:OD, 0:OD]
                )
                nc.vector.tensor_scalar_mul(
                    out_sb[:, c, hh * OD : (hh + 1) * OD],
                    ps_o[:],
                    sinv[:, c, hh : hh + 1],
                )

        nc.sync.dma_start(outp_d.rearrange("(c p) d -> p c d", p=P), out_sb[:])

    nc.compile()
    return nc


def kernel(h, W, a):
    from concourse.bass_utils import run_bass_kernel_spmd

    h = np.ascontiguousarray(np.asarray(h, dtype=np.float32))
    W = np.ascontiguousarray(np.asarray(W, dtype=np.float32))
    a = np.ascontiguousarray(np.asarray(a, dtype=np.float32))

    if "nc" not in _cache:
        _cache["nc"] = _build()
    nc = _cache["nc"]

    in_maps = [
        {
            "h": h,
            "hi": np.ascontiguousarray(h[c * IS : (c + 1) * IS]),
            "W": W,
            "a": a,
        }
        for c in range(NCORES)
    ]
    res = run_bass_kernel_spmd(nc, in_maps, core_ids=list(range(NCORES)))
    _cache["last_results"] = res
    rs = res.results
    out = np.concatenate([r["outp"] for r in rs], axis=0)
    attn = np.concatenate(
        [np.stack([r[f"attn{hh}"] for hh in range(H)], axis=-1) for r in rs], axis=0
    )
    return out, attn


# revision 6
# speedup vs baseline: 1.2941x; 1.2941x over previous
"""Multi-head graph attention (GAT) Bass kernel for Trainium2, 8 NeuronCores.

Shards the destination-node (i) dimension across 8 cores. Each core:
  - computes Wh = h @ W (replicated; float32r matmuls), attention coefficients
    f_j for all nodes (per-partition columns) and f_i for its 512-row slice
    (broadcast rows),
  - computes P^T[j, i] = exp(leaky_relu(f_i[i] + f_j[j], 0.2)) tile by tile
    with j on partitions via two ScalarE passes (Prelu with per-partition
    bias f_j, then a batched Exp emitting float32r),
  - aggregates out^T[d, i] = sum_j Wh[j, d] * P^T[j, i] on the PE in float32r
    with an appended ones column whose output row is the softmax
    denominator S,
  - scales P^T by 1/S (tensor_tensor against a partition-broadcast 1/S row)
    and stores per-head TRANSPOSED attention planes attnT_h[j, i] with fully
    contiguous DMAs.
The host transposes the per-head planes, stacks heads, and concatenates the
per-core row slices (pure memory reordering, no arithmetic).
"""

import numpy as np

N = 4096
D = 256
H = 4
OD = 64
NCORES = 8
IS = N // NCORES  # 512 rows of i per core
P = 128
NJC = N // P  # 32 j-chunks
NIC = IS // P  # 4 i-chunks per core

_cache = {}


def _build():
    from contextlib import ExitStack

    import concourse.bacc as bacc
    import concourse.mybir as mybir
    import concourse.tile as tile
    from concourse import masks

    dt = mybir.dt
    AF = mybir.ActivationFunctionType

    nc = bacc.Bacc("TRN2", target_bir_lowering=False, debug=False)

    h_d = nc.dram_tensor("h", [N, D], dt.float32, kind="ExternalInput").ap()
    hi_d = nc.dram_tensor("hi", [IS, D], dt.float32, kind="ExternalInput").ap()
    w_d = nc.dram_tensor("W", [D, D], dt.float32, kind="ExternalInput").ap()
    a_d = nc.dram_tensor("a", [H, 2 * OD], dt.float32, kind="ExternalInput").ap()

    attn_d = [
        nc.dram_tensor(f"attnT{hh}", [N, IS], dt.float32, kind="ExternalOutput").ap()
        for hh in range(H)
    ]
    outp_d = nc.dram_tensor("outp", [IS, D], dt.float32, kind="ExternalOutput").ap()

    with tile.TileContext(nc) as tc, ExitStack() as ctx:
        # ---------------- persistent small tensors ----------------
        const_pool = ctx.enter_context(tc.tile_pool(name="const", bufs=1))
        ident_f = const_pool.tile([P, P], dt.float32)
        masks.make_identity(nc, ident_f[:])
        ones_f = const_pool.tile([P, 1], dt.float32)
        nc.vector.memset(ones_f[:], 1.0)

        # AW[p, jc, h, 0:64] = Wh[jc*128+p, h*64:(h+1)*64]; AW[..., 64] = 1.0
        aw = const_pool.tile([P, NJC, H, OD + 1], dt.float32r)
        fj = const_pool.tile([P, NJC, H], dt.float32)
        fib = const_pool.tile([P, H, IS], dt.float32)
        sinv = const_pool.tile([P, NIC, H], dt.float32)
        out_sb = const_pool.tile([P, NIC, D], dt.float32)

        # ---------------- preamble (freed before the main loop) -------------
        with tc.tile_pool(name="pre", bufs=1) as pre, tc.tile_pool(
            name="pre_ps", bufs=2, space="PSUM"
        ) as pre_ps:
            hi_sb = pre.tile([P, NIC, D], dt.float32)
            nc.sync.dma_start(hi_sb[:], hi_d.rearrange("(k p) d -> p k d", p=P))
            w_sb = pre.tile([P, 2, D], dt.float32)
            nc.sync.dma_start(w_sb[:], w_d.rearrange("(k p) c -> p k c", p=P))
            h_sb = pre.tile([P, NJC, D], dt.float32)
            nc.sync.dma_start(h_sb[:], h_d.rearrange("(k p) d -> p k d", p=P))

            # block-diagonal attention vectors: A_*_BD[64*(hh%2):, hh//2, hh]
            a_dst = pre.tile([P, 2, H], dt.float32)
            a_src = pre.tile([P, 2, H], dt.float32)
            nc.vector.memset(a_dst[:], 0.0)
            nc.vector.memset(a_src[:], 0.0)
            for hh in range(H):
                ps = (hh % 2) * OD
                dc = hh // 2
                nc.sync.dma_start(
                    a_dst[ps : ps + OD, dc, hh : hh + 1],
                    a_d[hh : hh + 1, 0:OD].rearrange("o d -> d o"),
                )
                nc.sync.dma_start(
                    a_src[ps : ps + OD, dc, hh : hh + 1],
                    a_d[hh : hh + 1, OD : 2 * OD].rearrange("o d -> d o"),
                )

            # --- small path first: f_i rows for this core's slice -> fib ---
            hi_t = pre.tile([P, 2, IS], dt.float32)
            for kt in range(NIC):
                for dc in range(2):
                    tp = pre_ps.tile([P, P], dt.float32, tag="pre_tp")
                    nc.tensor.transpose(
                        tp[:], hi_sb[:, kt, dc * P : (dc + 1) * P], ident_f[:]
                    )
                    nc.vector.tensor_copy(hi_t[:, dc, kt * P : (kt + 1) * P], tp[:])
            whi_t = pre.tile([P, 2, IS], dt.float32)
            for cc in range(2):
                ps_w = pre_ps.tile([P, 512], dt.float32, tag="pre_mm")
                for dc in range(2):
                    nc.tensor.matmul(
                        ps_w[:],
                        w_sb[:, dc, cc * P : (cc + 1) * P],
                        hi_t[:, dc, :],
                        start=(dc == 0),
                        stop=(dc == 1),
                    )
                nc.vector.tensor_copy(whi_t[:, cc, :], ps_w[:])
            for hh in range(H):
                ps_f = pre_ps.tile([1, IS], dt.float32, tag="pre_fi")
                for dc in range(2):
                    nc.tensor.matmul(
                        ps_f[:],
                        a_dst[:, dc, hh : hh + 1],
                        whi_t[:, dc, :],
                        start=(dc == 0),
                        stop=(dc == 1),
                    )
                fi_row = pre.tile([1, IS], dt.float32, tag="fi_row")
                nc.vector.tensor_copy(fi_row[:], ps_f[:])
                nc.gpsimd.partition_broadcast(fib[:, hh, :], fi_row[:])

            # --- full-graph path: hT (f32r), WhT, f_j, AW ---
            w_r = pre.tile([P, 2, D], dt.float32r)
            nc.vector.tensor_copy(w_r[:], w_sb[:])
            h_rt = pre.tile([P, 2, N], dt.float32r)
            for kt in range(NJC):
                for dc in range(2):
                    tp = pre_ps.tile([P, P], dt.float32, tag="pre_tp")
                    nc.tensor.transpose(
                        tp[:], h_sb[:, kt, dc * P : (dc + 1) * P], ident_f[:]
                    )
                    nc.vector.tensor_copy(h_rt[:, dc, kt * P : (kt + 1) * P], tp[:])

            wh_t = pre.tile([P, 2, N], dt.float32)
            for ng in range(8):
                for cc in range(2):
                    ps_w = pre_ps.tile([P, 512], dt.float32, tag="pre_mm")
                    for dc in range(2):
                        nc.tensor.matmul(
                            ps_w[:],
                            w_r[:, dc, cc * P : (cc + 1) * P],
                            h_rt[:, dc, ng * 512 : (ng + 1) * 512],
                            start=(dc == 0),
                            stop=(dc == 1),
                        )
                    nc.vector.tensor_copy(
                        wh_t[:, cc, ng * 512 : (ng + 1) * 512], ps_w[:]
                    )

            # f_j columns (all nodes)
            for jc in range(NJC):
                ps_f = pre_ps.tile([P, H], dt.float32, tag="pre_f")
                for dc in range(2):
                    nc.tensor.matmul(
                        ps_f[:],
                        wh_t[:, dc, jc * P : (jc + 1) * P],
                        a_src[:, dc, :],
                        start=(dc == 0),
                        stop=(dc == 1),
                    )
                nc.vector.tensor_copy(fj[:, jc, :], ps_f[:])

            # AW from WhT via PE transposes (rounded to f32r on evac)
            for jc in range(NJC):
                for cc in range(2):
                    tp = pre_ps.tile([P, P], dt.float32, tag="pre_tp")
                    nc.tensor.transpose(
                        tp[:], wh_t[:, cc, jc * P : (jc + 1) * P], ident_f[:]
                    )
                    nc.vector.tensor_copy(aw[:, jc, 2 * cc : 2 * cc + 2, 0:OD], tp[:])
            nc.vector.tensor_copy(
                aw[:, :, :, OD : OD + 1],
                ones_f[:].to_broadcast([P, NJC, H, 1]),
            )

        # ---------------- main loop ----------------
        et_pool = ctx.enter_context(tc.tile_pool(name="et", bufs=3))
        pt_pool = ctx.enter_context(tc.tile_pool(name="pt", bufs=10))
        sc_pool = ctx.enter_context(tc.tile_pool(name="sc", bufs=4))
        agg_ps_pool = ctx.enter_context(
            tc.tile_pool(name="agg_ps", bufs=2, space="PSUM")
        )
        sm_ps_pool = ctx.enter_context(tc.tile_pool(name="sm_ps", bufs=2, space="PSUM"))
        sm_sb_pool = ctx.enter_context(tc.tile_pool(name="sm_sb", bufs=2))

        NJG = NJC // 4  # 8 groups of 4 j-chunks
        for hh in range(H):
            agg_ps = agg_ps_pool.tile([OD + 1, 512], dt.float32)
            pt_tiles = []
            for jg in range(NJG):
                et = et_pool.tile([P, 4, 512], dt.float32)
                for u in range(4):
                    jc = jg * 4 + u
                    nc.scalar.activation(
                        et[:, u, :],
                        fib[:, hh, :],
                        AF.Prelu,
                        bias=fj[:, jc, hh : hh + 1],
                        scale=1.0,
                        alpha=0.2,
                    )
                pt = pt_pool.tile(
                    [P, 4, 512], dt.float32r, name=f"pt{hh}_{jg}", tag="pt"
                )
                nc.scalar.activation(pt[:], et[:], AF.Exp)
                pt_tiles.append(pt)
                for u in range(4):
                    jc = jg * 4 + u
                    nc.tensor.matmul(
                        agg_ps[:],
                        aw[:, jc, hh, :],
                        pt[:, u, :],
                        start=(jc == 0),
                        stop=(jc == NJC - 1),
                    )

            # softmax denominators: S row -> 1/S broadcast + 1/S columns
            s_row = sm_sb_pool.tile([1, 512], dt.float32, tag="s_row")
            nc.vector.tensor_copy(s_row[:], agg_ps[OD : OD + 1, :])
            srinv = sm_sb_pool.tile([1, 512], dt.float32, tag="srinv")
            nc.vector.reciprocal(srinv[:], s_row[:])
            sinv_b = sm_sb_pool.tile([P, 512], dt.float32, tag="sinv_b")
            nc.gpsimd.partition_broadcast(sinv_b[:], srinv[:])
            for c in range(NIC):
                ps_s = sm_ps_pool.tile([P, 1], dt.float32, tag="ps_s")
                nc.tensor.transpose(
                    ps_s[:], srinv[0:1, c * P : (c + 1) * P], ident_f[0:1, 0:1]
                )
                nc.vector.tensor_copy(sinv[:, c, hh : hh + 1], ps_s[:])

            # scale P^T rows by 1/S[i] and store transposed attention planes
            for jg in range(NJG):
                sc = sc_pool.tile([P, 4, 512], dt.float32)
                nc.vector.tensor_mul(
                    sc[:],
                    pt_tiles[jg][:],
                    sinv_b[:].unsqueeze(1).to_broadcast([P, 4, 512]),
                )
                nc.sync.dma_start(
                    attn_d[hh][jg * 512 : (jg + 1) * 512, :].rearrange(
                        "(u p) i -> p u i", p=P
                    ),
                    sc[:],
                )

            # aggregated output: transpose [65,512] psum back to i-major
            o_sb = sm_sb_pool.tile([OD + 1, 512], dt.float32, tag="o_sb")
            nc.vector.tensor_copy(o_sb[:], agg_ps[:])
            for c in range(NIC):
                ps_o = sm_ps_pool.tile([P, OD], dt.float32, tag="ps_o")
                nc.tensor.transpose(
                    ps_o[:], o_sb[0:OD, c * P : (c + 1) * P], ident_f[0:OD, 0:OD]
                )
                nc.vector.tensor_scalar_mul(
                    out_sb[:, c, hh * OD : (hh + 1) * OD],
                    ps_o[:],
                    sinv[:, c, hh : hh + 1],
                )

        nc.sync.dma_start(outp_d.rearrange("(c p) d -> p c d", p=P), out_sb[:])

    nc.compile()
    return nc


def kernel(h, W, a):
    from concourse.bass_utils import run_bass_kernel_spmd

    h = np.ascontiguousarray(np.asarray(h, dtype=np.float32))
    W = np.ascontiguousarray(np.asarray(W, dtype=np.float32))
    a = np.ascontiguousarray(np.asarray(a, dtype=np.float32))

    if "nc" not in _cache:
        _cache["nc"] = _build()
    nc = _cache["nc"]

    in_maps = [
        {
            "h": h,
            "hi": np.ascontiguousarray(h[c * IS : (c + 1) * IS]),
            "W": W,
            "a": a,
        }
        for c in range(NCORES)
    ]
    res = run_bass_kernel_spmd(nc, in_maps, core_ids=list(range(NCORES)))
    _cache["last_results"] = res
    rs = res.results

    out = np.concatenate([r["outp"] for r in rs], axis=0)
    attn = np.empty((N, N, H), dtype=np.float32)
    for c, r in enumerate(rs):
        for hh in range(H):
            attn[c * IS : (c + 1) * IS, :, hh] = r[f"attnT{hh}"].T
    return out, attn


# revision 8
# speedup vs baseline: 1.2946x; 1.0004x over previous
"""Multi-head graph attention (GAT) Bass kernel for Trainium2, 8 NeuronCores.

Shards the destination-node (i) dimension across 8 cores. Each core:
  - transposes h on the PE, then computes [Wh | f_j] with fused float32r
    matmuls against the combined [W | W @ a_src] moving operand (one weight
    load per h^T chunk yields both the aggregation weights and the source
    attention coefficients),
  - computes f_i for its 512-row slice via W @ a_dst and broadcasts it across
    partitions,
  - computes P^T[j, i] = exp(leaky_relu(f_i[i] + f_j[j], 0.2)) with j on
    partitions: ScalarE Prelu with per-partition bias f_j (one quarter of the
    chunks take a 3-instruction VectorE leaky-relu path instead), then a
    batched ScalarE Exp emitting float32r,
  - aggregates out^T[d, i] = sum_j Wh[j, d] * P^T[j, i] on the PE in float32r
    with an appended ones column whose output row is the softmax
    denominator S,
  - scales P^T by 1/S (tensor_tensor against a partition-broadcast 1/S row,
    alternating VectorE / GpSimd) and stores per-head TRANSPOSED attention
    planes attnT_h[j, i] with fully contiguous DMAs.
The host transposes the per-head planes, stacks heads, and concatenates the
per-core row slices (pure memory reordering, no arithmetic).
"""

import numpy as np

N = 4096
D = 256
H = 4
OD = 64
NCORES = 8
IS = N // NCORES  # 512 rows of i per core
P = 128
NJC = N // P  # 32 j-chunks
NIC = IS // P  # 4 i-chunks per core

_cache = {}


def _build():
    from contextlib import ExitStack

    import concourse.bacc as bacc
    import concourse.mybir as mybir
    import concourse.tile as tile
    from concourse import masks

    dt = mybir.dt
    AF = mybir.ActivationFunctionType
    ALU = mybir.AluOpType

    nc = bacc.Bacc("TRN2", target_bir_lowering=False, debug=False)

    h_d = nc.dram_tensor("h", [N, D], dt.float32, kind="ExternalInput").ap()
    hi_d = nc.dram_tensor("hi", [IS, D], dt.float32, kind="ExternalInput").ap()
    w_d = nc.dram_tensor("W", [D, D], dt.float32, kind="ExternalInput").ap()
    a_d = nc.dram_tensor("a", [H, 2 * OD], dt.float32, kind="ExternalInput").ap()

    attn_d = [
        nc.dram_tensor(f"attnT{hh}", [N, IS], dt.float32, kind="ExternalOutput").ap()
        for hh in range(H)
    ]
    outp_d = nc.dram_tensor("outp", [IS, D], dt.float32, kind="ExternalOutput").ap()

    with tile.TileContext(nc) as tc, ExitStack() as ctx:
        # ---------------- persistent small tensors ----------------
        const_pool = ctx.enter_context(tc.tile_pool(name="const", bufs=1))
        ident_f = const_pool.tile([P, P], dt.float32)
        masks.make_identity(nc, ident_f[:])
        ones_f = const_pool.tile([P, 1], dt.float32)
        nc.vector.memset(ones_f[:], 1.0)

        # AW[p, jc, h, 0:64] = Wh[jc*128+p, h*64:(h+1)*64]; AW[..., 64] = 1.0
        aw = const_pool.tile([P, NJC, H, OD + 1], dt.float32r)
        fj = const_pool.tile([P, NJC, H], dt.float32)
        fib = const_pool.tile([P, H, IS], dt.float32)
        sinv = const_pool.tile([P, NIC, H], dt.float32)
        out_sb = const_pool.tile([P, NIC, D], dt.float32)

        # ---------------- preamble (freed before the main loop) -------------
        with tc.tile_pool(name="pre", bufs=1) as pre, tc.tile_pool(
            name="pre_ps", bufs=2, space="PSUM"
        ) as pre_ps:
            hi_sb = pre.tile([P, NIC, D], dt.float32)
            nc.sync.dma_start(hi_sb[:], hi_d.rearrange("(k p) d -> p k d", p=P))
            w_sb = pre.tile([P, 2, D], dt.float32)
            nc.sync.dma_start(w_sb[:], w_d.rearrange("(k p) c -> p k c", p=P))
            h_sb = pre.tile([P, NJC, D], dt.float32)
            nc.sync.dma_start(h_sb[:], h_d.rearrange("(k p) d -> p k d", p=P))

            # block-diagonal attention vectors: A_*[64*(hh%2):, hh//2, hh]
            a_dst = pre.tile([P, 2, H], dt.float32)
            a_src = pre.tile([P, 2, H], dt.float32)
            nc.vector.memset(a_dst[:], 0.0)
            nc.vector.memset(a_src[:], 0.0)
            for hh in range(H):
                ps = (hh % 2) * OD
                dc = hh // 2
                nc.sync.dma_start(
                    a_dst[ps : ps + OD, dc, hh : hh + 1],
                    a_d[hh : hh + 1, 0:OD].rearrange("o d -> d o"),
                )
                nc.sync.dma_start(
                    a_src[ps : ps + OD, dc, hh : hh + 1],
                    a_d[hh : hh + 1, OD : 2 * OD].rearrange("o d -> d o"),
                )

            # W^T (for v* = W @ a_*), then va/vd [d, h]
            w_t = pre.tile([P, 2, D], dt.float32)
            for cc in range(2):
                for dc in range(2):
                    tp = pre_ps.tile([P, P], dt.float32, tag="pre_tp")
                    nc.tensor.transpose(
                        tp[:], w_sb[:, dc, cc * P : (cc + 1) * P], ident_f[:]
                    )
                    nc.vector.tensor_copy(w_t[:, cc, dc * P : (dc + 1) * P], tp[:])

            # wva = [W | va] as the fused f32r moving operand
            wva = pre.tile([P, 2, D + H], dt.float32r)
            nc.vector.tensor_copy(wva[:, :, 0:D], w_sb[:])
            vd_sb = pre.tile([P, 2, H], dt.float32)
            for dc in range(2):
                ps_v = pre_ps.tile([P, H], dt.float32, tag="pre_sm")
                for cc in range(2):
                    nc.tensor.matmul(
                        ps_v[:],
                        w_t[:, cc, dc * P : (dc + 1) * P],
                        a_src[:, cc, :],
                        start=(cc == 0),
                        stop=(cc == 1),
                    )
                nc.vector.tensor_copy(wva[:, dc, D : D + H], ps_v[:])
                ps_v2 = pre_ps.tile([P, H], dt.float32, tag="pre_sm")
                for cc in range(2):
                    nc.tensor.matmul(
                        ps_v2[:],
                        w_t[:, cc, dc * P : (dc + 1) * P],
                        a_dst[:, cc, :],
                        start=(cc == 0),
                        stop=(cc == 1),
                    )
                nc.vector.tensor_copy(vd_sb[:, dc, :], ps_v2[:])

            # f_i rows for this core's slice -> partition-broadcast tiles
            hi_t = pre.tile([P, 2, IS], dt.float32)
            for kt in range(NIC):
                for dc in range(2):
                    tp = pre_ps.tile([P, P], dt.float32, tag="pre_tp")
                    nc.tensor.transpose(
                        tp[:], hi_sb[:, kt, dc * P : (dc + 1) * P], ident_f[:]
                    )
                    nc.vector.tensor_copy(hi_t[:, dc, kt * P : (kt + 1) * P], tp[:])
            for hh in range(H):
                ps_f = pre_ps.tile([1, IS], dt.float32, tag="pre_fi")
                for dc in range(2):
                    nc.tensor.matmul(
                        ps_f[:],
                        vd_sb[:, dc, hh : hh + 1],
                        hi_t[:, dc, :],
                        start=(dc == 0),
                        stop=(dc == 1),
                    )
                fi_row = pre.tile([1, IS], dt.float32, tag="fi_row")
                nc.vector.tensor_copy(fi_row[:], ps_f[:])
                nc.gpsimd.partition_broadcast(fib[:, hh, :], fi_row[:])

            # per h^T chunk: transpose, then one fused matmul -> [Wh | f_j]
            h_rt = pre.tile([P, 2, N], dt.float32r)
            for kt in range(NJC):
                for dc in range(2):
                    tp = pre_ps.tile([P, P], dt.float32, tag="pre_tp")
                    nc.tensor.transpose(
                        tp[:], h_sb[:, kt, dc * P : (dc + 1) * P], ident_f[:]
                    )
                    nc.vector.tensor_copy(h_rt[:, dc, kt * P : (kt + 1) * P], tp[:])
                ps_wh = pre_ps.tile([P, D + H], dt.float32, tag="pre_wh")
                for dc in range(2):
                    nc.tensor.matmul(
                        ps_wh[:],
                        h_rt[:, dc, kt * P : (kt + 1) * P],
                        wva[:, dc, :],
                        start=(dc == 0),
                        stop=(dc == 1),
                    )
                nc.vector.tensor_copy(
                    aw[:, kt, :, 0:OD],
                    ps_wh[:, 0:D].rearrange("p (hh o) -> p hh o", o=OD),
                )
                nc.vector.tensor_copy(fj[:, kt, :], ps_wh[:, D : D + H])

            nc.vector.tensor_copy(
                aw[:, :, :, OD : OD + 1],
                ones_f[:].to_broadcast([P, NJC, H, 1]),
            )

        # ---------------- main loop ----------------
        et_pool = ctx.enter_context(tc.tile_pool(name="et", bufs=3))
        dve_tmp_pool = ctx.enter_context(tc.tile_pool(name="dve_tmp", bufs=2))
        pt_pool = ctx.enter_context(tc.tile_pool(name="pt", bufs=10))
        sc_pool = ctx.enter_context(tc.tile_pool(name="sc", bufs=4))
        agg_ps_pool = ctx.enter_context(
            tc.tile_pool(name="agg_ps", bufs=2, space="PSUM")
        )
        sm_ps_pool = ctx.enter_context(tc.tile_pool(name="sm_ps", bufs=2, space="PSUM"))
        sm_sb_pool = ctx.enter_context(tc.tile_pool(name="sm_sb", bufs=1))

        NJG = NJC // 4  # 8 groups of 4 j-chunks
        for hh in range(H):
            agg_ps = agg_ps_pool.tile([OD + 1, 512], dt.float32)
            pt_tiles = []
            for jg in range(NJG):
                et = et_pool.tile([P, 4, 512], dt.float32)
                for u in range(4):
                    jc = jg * 4 + u
                    if u < 3:
                        nc.scalar.activation(
                            et[:, u, :],
                            fib[:, hh, :],
                            AF.Prelu,
                            bias=fj[:, jc, hh : hh + 1],
                            scale=1.0,
                            alpha=0.2,
                        )
                    else:
                        # VectorE leaky-relu: y = x + fj; et = max(y, 0.2 y)
                        y = dve_tmp_pool.tile([P, 512], dt.float32, tag="lr_y")
                        nc.vector.tensor_scalar_add(
                            y[:], fib[:, hh, :], fj[:, jc, hh : hh + 1]
                        )
                        y2 = dve_tmp_pool.tile([P, 512], dt.float32, tag="lr_y2")
                        nc.vector.tensor_scalar(
                            y2[:],
                            fib[:, hh, :],
                            fj[:, jc, hh : hh + 1],
                            0.2,
                            ALU.add,
                            ALU.mult,
                        )
                        nc.vector.tensor_tensor(
                            et[:, u, :], y[:], y2[:], ALU.max
                        )
                pt = pt_pool.tile(
                    [P, 4, 512], dt.float32r, name=f"pt{hh}_{jg}", tag="pt"
                )
                nc.scalar.activation(pt[:], et[:], AF.Exp)
                pt_tiles.append(pt)
                for u in range(4):
                    jc = jg * 4 + u
                    nc.tensor.matmul(
                        agg_ps[:],
                        aw[:, jc, hh, :],
                        pt[:, u, :],
                        start=(jc == 0),
                        stop=(jc == NJC - 1),
                    )

            # softmax denominators: S row -> broadcast -> 1/S (NR approx)
            s_row = sm_sb_pool.tile([1, 512], dt.float32, tag="s_row")
            nc.vector.tensor_copy(s_row[:], agg_ps[OD : OD + 1, :])
            s_b = sm_sb_pool.tile([P, 512], dt.float32, tag="s_b")
            nc.gpsimd.partition_broadcast(s_b[:], s_row[:])
            sinv_b = sm_sb_pool.tile([P, 512], dt.float32, tag="sinv_b")
            rec_tmp = sm_sb_pool.tile([P, 512], dt.float32, tag="rec_tmp")
            nc.vector.reciprocal_approx_accurate(sinv_b[:], s_b[:], rec_tmp[:])
            # 1/S columns for the out epilogue
            for c in range(NIC):
                ps_s = sm_ps_pool.tile([P, 1], dt.float32, tag="ps_s")
                nc.tensor.transpose(
                    ps_s[:], s_row[0:1, c * P : (c + 1) * P], ident_f[0:1, 0:1]
                )
                nc.vector.reciprocal(sinv[:, c, hh : hh + 1], ps_s[:])

            # scale P^T rows by 1/S[i] and store transposed attention planes
            for jg in range(NJG):
                sc = sc_pool.tile([P, 4, 512], dt.float32)
                eng = nc.vector if jg % 2 == 0 else nc.gpsimd
                eng.tensor_mul(
                    sc[:],
                    pt_tiles[jg][:],
                    sinv_b[:].unsqueeze(1).to_broadcast([P, 4, 512]),
                )
                nc.sync.dma_start(
                    attn_d[hh][jg * 512 : (jg + 1) * 512, :].rearrange(
                        "(u p) i -> p u i", p=P
                    ),
                    sc[:],
                )

            # aggregated output: transpose [65,512] psum back to i-major
            o_sb = sm_sb_pool.tile([OD + 1, 512], dt.float32, tag="o_sb")
            nc.vector.tensor_copy(o_sb[:], agg_ps[:])
            for c in range(NIC):
                ps_o = sm_ps_pool.tile([P, OD], dt.float32, tag="ps_o")
                nc.tensor.transpose(
                    ps_o[:], o_sb[0:OD, c * P : (c + 1) * P], ident_f[0:OD, 0:OD]
                )
                nc.vector.tensor_scalar_mul(
                    out_sb[:, c, hh * OD : (hh + 1) * OD],
                    ps_o[:],
                    sinv[:, c, hh : hh + 1],
                )

        nc.sync.dma_start(outp_d.rearrange("(c p) d -> p c d", p=P), out_sb[:])

    nc.compile()
    return nc


def kernel(h, W, a):
    from concourse.bass_utils import run_bass_kernel_spmd

    h = np.ascontiguousarray(np.asarray(h, dtype=np.float32))
    W = np.ascontiguousarray(np.asarray(W, dtype=np.float32))
    a = np.ascontiguousarray(np.asarray(a, dtype=np.float32))

    if "nc" not in _cache:
        _cache["nc"] = _build()
    nc = _cache["nc"]

    in_maps = [
        {
            "h": h,
            "hi": np.ascontiguousarray(h[c * IS : (c + 1) * IS]),
            "W": W,
            "a": a,
        }
        for c in range(NCORES)
    ]
    res = run_bass_kernel_spmd(nc, in_maps, core_ids=list(range(NCORES)))
    _cache["last_results"] = res
    rs = res.results

    out = np.concatenate([r["outp"] for r in rs], axis=0)
    attn = np.empty((N, N, H), dtype=np.float32)
    for c, r in enumerate(rs):
        for hh in range(H):
            attn[c * IS : (c + 1) * IS, :, hh] = r[f"attnT{hh}"].T
    return out, attn


# revision 10
# speedup vs baseline: 1.3101x; 1.0120x over previous
"""Multi-head graph attention (GAT) Bass kernel for Trainium2, 8 NeuronCores.

Shards the destination-node (i) dimension across 8 cores. Each core:
  - transposes h on the PE, then computes [Wh | f_j] with fused float32r
    matmuls against the combined [W | W @ a_src] moving operand (one weight
    load per h^T chunk yields both the aggregation weights and the source
    attention coefficients),
  - computes f_i for its 512-row slice via W @ a_dst and broadcasts it across
    partitions,
  - computes P^T[j, i] = exp(leaky_relu(f_i[i] + f_j[j], 0.2)) with j on
    partitions: ScalarE Prelu with per-partition bias f_j (one quarter of the
    chunks take a 3-instruction VectorE leaky-relu path instead), then a
    batched ScalarE Exp emitting float32r,
  - aggregates out^T[d, i] = sum_j Wh[j, d] * P^T[j, i] on the PE in float32r
    with an appended ones column whose output row is the softmax
    denominator S,
  - scales P^T by 1/S (tensor_tensor against a partition-broadcast 1/S row,
    alternating VectorE / GpSimd) and stores per-head TRANSPOSED attention
    planes attnT_h[j, i] with fully contiguous DMAs.
The host transposes the per-head planes, stacks heads, and concatenates the
per-core row slices (pure memory reordering, no arithmetic).
"""

import numpy as np

N = 4096
D = 256
H = 4
OD = 64
NCORES = 8
IS = N // NCORES  # 512 rows of i per core
P = 128
NJC = N // P  # 32 j-chunks
NIC = IS // P  # 4 i-chunks per core

_cache = {}


def _build():
    from contextlib import ExitStack

    import concourse.bacc as bacc
    import concourse.mybir as mybir
    import concourse.tile as tile
    from concourse import masks

    dt = mybir.dt
    AF = mybir.ActivationFunctionType
    ALU = mybir.AluOpType

    nc = bacc.Bacc("TRN2", target_bir_lowering=False, debug=False)

    h_d = nc.dram_tensor("h", [N, D], dt.float32, kind="ExternalInput").ap()
    hi_d = nc.dram_tensor("hi", [IS, D], dt.float32, kind="ExternalInput").ap()
    w_d = nc.dram_tensor("W", [D, D], dt.float32, kind="ExternalInput").ap()
    a_d = nc.dram_tensor("a", [H, 2 * OD], dt.float32, kind="ExternalInput").ap()

    attn_d = [
        nc.dram_tensor(f"attnT{hh}", [N, IS], dt.float32, kind="ExternalOutput").ap()
        for hh in range(H)
    ]
    outp_d = nc.dram_tensor("outp", [IS, D], dt.float32, kind="ExternalOutput").ap()

    with tile.TileContext(nc) as tc, ExitStack() as ctx:
        # ---------------- persistent small tensors ----------------
        const_pool = ctx.enter_context(tc.tile_pool(name="const", bufs=1))
        ident_f = const_pool.tile([P, P], dt.float32)
        masks.make_identity(nc, ident_f[:])
        ones_f = const_pool.tile([P, 1], dt.float32)
        nc.vector.memset(ones_f[:], 1.0)

        # AW[p, jc, h, 0:64] = Wh[jc*128+p, h*64:(h+1)*64]; AW[..., 64] = 1.0
        aw = const_pool.tile([P, NJC, H, OD + 1], dt.float32r)
        fj = const_pool.tile([P, NJC, H], dt.float32)
        fib = const_pool.tile([P, H, IS], dt.float32)
        sinv = const_pool.tile([P, NIC, H], dt.float32)
        out_sb = const_pool.tile([P, NIC, D], dt.float32)

        # ---------------- preamble (freed before the main loop) -------------
        with tc.tile_pool(name="pre", bufs=1) as pre, tc.tile_pool(
            name="pre_ps", bufs=2, space="PSUM"
        ) as pre_ps:
            hi_sb = pre.tile([P, NIC, D], dt.float32)
            nc.sync.dma_start(hi_sb[:], hi_d.rearrange("(k p) d -> p k d", p=P))
            w_sb = pre.tile([P, 2, D], dt.float32)
            nc.sync.dma_start(w_sb[:], w_d.rearrange("(k p) c -> p k c", p=P))
            h_sb = pre.tile([P, NJC, D], dt.float32)
            h_re = h_d.rearrange("(k p) d -> p k d", p=P)
            for kg in range(8):
                nc.sync.dma_start(
                    h_sb[:, kg * 4 : (kg + 1) * 4, :], h_re[:, kg * 4 : (kg + 1) * 4, :]
                )

            # block-diagonal attention vectors: A_*[64*(hh%2):, hh//2, hh]
            a_dst = pre.tile([P, 2, H], dt.float32)
            a_src = pre.tile([P, 2, H], dt.float32)
            nc.vector.memset(a_dst[:], 0.0)
            nc.vector.memset(a_src[:], 0.0)
            for hh in range(H):
                ps = (hh % 2) * OD
                dc = hh // 2
                nc.sync.dma_start(
                    a_dst[ps : ps + OD, dc, hh : hh + 1],
                    a_d[hh : hh + 1, 0:OD].rearrange("o d -> d o"),
                )
                nc.sync.dma_start(
                    a_src[ps : ps + OD, dc, hh : hh + 1],
                    a_d[hh : hh + 1, OD : 2 * OD].rearrange("o d -> d o"),
                )

            # W^T (for v* = W @ a_*), then va/vd [d, h]
            w_t = pre.tile([P, 2, D], dt.float32)
            for cc in range(2):
                for dc in range(2):
                    tp = pre_ps.tile([P, P], dt.float32, tag="pre_tp", bufs=3)
                    nc.tensor.transpose(
                        tp[:], w_sb[:, dc, cc * P : (cc + 1) * P], ident_f[:]
                    )
                    nc.vector.tensor_copy(w_t[:, cc, dc * P : (dc + 1) * P], tp[:])

            # wva = [W | va] as the fused f32r moving operand
            wva = pre.tile([P, 2, D + H], dt.float32r)
            nc.vector.tensor_copy(wva[:, :, 0:D], w_sb[:])
            vd_sb = pre.tile([P, 2, H], dt.float32)
            for dc in range(2):
                ps_v = pre_ps.tile([P, H], dt.float32, tag="pre_sm")
                for cc in range(2):
                    nc.tensor.matmul(
                        ps_v[:],
                        w_t[:, cc, dc * P : (dc + 1) * P],
                        a_src[:, cc, :],
                        start=(cc == 0),
                        stop=(cc == 1),
                    )
                nc.vector.tensor_copy(wva[:, dc, D : D + H], ps_v[:])
                ps_v2 = pre_ps.tile([P, H], dt.float32, tag="pre_sm")
                for cc in range(2):
                    nc.tensor.matmul(
                        ps_v2[:],
                        w_t[:, cc, dc * P : (dc + 1) * P],
                        a_dst[:, cc, :],
                        start=(cc == 0),
                        stop=(cc == 1),
                    )
                nc.vector.tensor_copy(vd_sb[:, dc, :], ps_v2[:])

            # f_i rows for this core's slice -> partition-broadcast tiles
            hi_t = pre.tile([P, 2, IS], dt.float32)
            for kt in range(NIC):
                for dc in range(2):
                    tp = pre_ps.tile([P, P], dt.float32, tag="pre_tp", bufs=3)
                    nc.tensor.transpose(
                        tp[:], hi_sb[:, kt, dc * P : (dc + 1) * P], ident_f[:]
                    )
                    nc.vector.tensor_copy(hi_t[:, dc, kt * P : (kt + 1) * P], tp[:])
            for hh in range(H):
                ps_f = pre_ps.tile([1, IS], dt.float32, tag="pre_fi", bufs=1)
                for dc in range(2):
                    nc.tensor.matmul(
                        ps_f[:],
                        vd_sb[:, dc, hh : hh + 1],
                        hi_t[:, dc, :],
                        start=(dc == 0),
                        stop=(dc == 1),
                    )
                fi_row = pre.tile([1, IS], dt.float32, tag="fi_row")
                nc.vector.tensor_copy(fi_row[:], ps_f[:])
                nc.gpsimd.partition_broadcast(fib[:, hh, :], fi_row[:])

            # per h^T chunk: transpose, then one fused matmul -> [Wh | f_j]
            h_rt = pre.tile([P, 2, N], dt.float32r)
            for kt in range(NJC):
                for dc in range(2):
                    tp = pre_ps.tile([P, P], dt.float32, tag="pre_tp", bufs=3)
                    nc.tensor.transpose(
                        tp[:], h_sb[:, kt, dc * P : (dc + 1) * P], ident_f[:]
                    )
                    nc.vector.tensor_copy(h_rt[:, dc, kt * P : (kt + 1) * P], tp[:])
                ps_wh = pre_ps.tile([P, D + H], dt.float32, tag="pre_wh")
                for dc in range(2):
                    nc.tensor.matmul(
                        ps_wh[:],
                        h_rt[:, dc, kt * P : (kt + 1) * P],
                        wva[:, dc, :],
                        start=(dc == 0),
                        stop=(dc == 1),
                    )
                nc.vector.tensor_copy(
                    aw[:, kt, :, 0:OD],
                    ps_wh[:, 0:D].rearrange("p (hh o) -> p hh o", o=OD),
                )
                nc.vector.tensor_copy(fj[:, kt, :], ps_wh[:, D : D + H])

            nc.vector.tensor_copy(
                aw[:, :, :, OD : OD + 1],
                ones_f[:].to_broadcast([P, NJC, H, 1]),
            )

        # ---------------- main loop ----------------
        et_pool = ctx.enter_context(tc.tile_pool(name="et", bufs=3))
        pt_pool = ctx.enter_context(tc.tile_pool(name="pt", bufs=10))
        sc_pool = ctx.enter_context(tc.tile_pool(name="sc", bufs=4))
        agg_ps_pool = ctx.enter_context(
            tc.tile_pool(name="agg_ps", bufs=2, space="PSUM")
        )
        sm_ps_pool = ctx.enter_context(tc.tile_pool(name="sm_ps", bufs=2, space="PSUM"))
        sm_sb_pool = ctx.enter_context(tc.tile_pool(name="sm_sb", bufs=1))

        NJG = NJC // 4  # 8 groups of 4 j-chunks
        for hh in range(H):
            agg_ps = agg_ps_pool.tile([OD + 1, 512], dt.float32)
            pt_tiles = []
            for jg in range(NJG):
                et = et_pool.tile([P, 4, 512], dt.float32)
                for u in range(4):
                    jc = jg * 4 + u
                    nc.scalar.activation(
                        et[:, u, :],
                        fib[:, hh, :],
                        AF.Prelu,
                        bias=fj[:, jc, hh : hh + 1],
                        scale=1.0,
                        alpha=0.2,
                    )
                pt = pt_pool.tile(
                    [P, 4, 512], dt.float32r, name=f"pt{hh}_{jg}", tag="pt"
                )
                nc.scalar.activation(pt[:], et[:], AF.Exp)
                pt_tiles.append(pt)
                for u in range(4):
                    jc = jg * 4 + u
                    nc.tensor.matmul(
                        agg_ps[:],
                        aw[:, jc, hh, :],
                        pt[:, u, :],
                        start=(jc == 0),
                        stop=(jc == NJC - 1),
                    )

            # softmax denominators: S row -> broadcast -> 1/S (NR approx)
            s_row = sm_sb_pool.tile([1, 512], dt.float32, tag="s_row")
            nc.vector.tensor_copy(s_row[:], agg_ps[OD : OD + 1, :])
            s_b = sm_sb_pool.tile([P, 512], dt.float32, tag="s_b")
            nc.gpsimd.partition_broadcast(s_b[:], s_row[:])
            sinv_b = sm_sb_pool.tile([P, 512], dt.float32, tag="sinv_b")
            rec_tmp = sm_sb_pool.tile([P, 512], dt.float32, tag="rec_tmp")
            nc.vector.reciprocal_approx_accurate(sinv_b[:], s_b[:], rec_tmp[:])
            # 1/S columns for the out epilogue
            for c in range(NIC):
                ps_s = sm_ps_pool.tile([P, 1], dt.float32, tag="ps_s")
                nc.tensor.transpose(
                    ps_s[:], s_row[0:1, c * P : (c + 1) * P], ident_f[0:1, 0:1]
                )
                nc.vector.reciprocal(sinv[:, c, hh : hh + 1], ps_s[:])

            # scale P^T rows by 1/S[i] and store transposed attention planes
            for jg in range(NJG):
                sc = sc_pool.tile([P, 4, 512], dt.float32)
                eng = nc.gpsimd if jg in (1, 4, 6) else nc.vector
                eng.tensor_mul(
                    sc[:],
                    pt_tiles[jg][:],
                    sinv_b[:].unsqueeze(1).to_broadcast([P, 4, 512]),
                )
                nc.sync.dma_start(
                    attn_d[hh][jg * 512 : (jg + 1) * 512, :].rearrange(
                        "(u p) i -> p u i", p=P
                    ),
                    sc[:],
                )

            # aggregated output: transpose [65,512] psum back to i-major
            o_sb = sm_sb_pool.tile([OD + 1, 512], dt.float32, tag="o_sb")
            nc.vector.tensor_copy(o_sb[:], agg_ps[:])
            for c in range(NIC):
                ps_o = sm_ps_pool.tile([P, OD], dt.float32, tag="ps_o")
                nc.tensor.transpose(
                    ps_o[:], o_sb[0:OD, c * P : (c + 1) * P], ident_f[0:OD, 0:OD]
                )
                nc.vector.tensor_scalar_mul(
                    out_sb[:, c, hh * OD : (hh + 1) * OD],
                    ps_o[:],
                    sinv[:, c, hh : hh + 1],
                )

        nc.sync.dma_start(outp_d.rearrange("(c p) d -> p c d", p=P), out_sb[:])

    nc.compile()
    return nc


def kernel(h, W, a):
    from concourse.bass_utils import run_bass_kernel_spmd

    h = np.ascontiguousarray(np.asarray(h, dtype=np.float32))
    W = np.ascontiguousarray(np.asarray(W, dtype=np.float32))
    a = np.ascontiguousarray(np.asarray(a, dtype=np.float32))

    if "nc" not in _cache:
        _cache["nc"] = _build()
    nc = _cache["nc"]

    in_maps = [
        {
            "h": h,
            "hi": np.ascontiguousarray(h[c * IS : (c + 1) * IS]),
            "W": W,
            "a": a,
        }
        for c in range(NCORES)
    ]
    res = run_bass_kernel_spmd(nc, in_maps, core_ids=list(range(NCORES)))
    _cache["last_results"] = res
    rs = res.results

    out = np.concatenate([r["outp"] for r in rs], axis=0)
    attn = np.empty((N, N, H), dtype=np.float32)
    for c, r in enumerate(rs):
        for hh in range(H):
            attn[c * IS : (c + 1) * IS, :, hh] = r[f"attnT{hh}"].T
    return out, attn


# revision 11
# speedup vs baseline: 1.4102x; 1.0764x over previous
"""Multi-head graph attention (GAT) Bass kernel for Trainium2, 8 NeuronCores.

Shards the destination-node (i) dimension across 8 cores. Each core:
  - transposes h on the PE, then computes [Wh | f_j] with fused float32r
    matmuls against the combined [W | W @ a_src] moving operand (one weight
    load per h^T chunk yields both the aggregation weights and the source
    attention coefficients),
  - computes f_i for its 512-row slice via W @ a_dst and broadcasts it across
    partitions,
  - computes P^T[j, i] = exp(leaky_relu(f_i[i] + f_j[j], 0.2)) with j on
    partitions: ScalarE Prelu with per-partition bias f_j (one quarter of the
    chunks take a 3-instruction VectorE leaky-relu path instead), then a
    batched ScalarE Exp emitting float32r,
  - aggregates out^T[d, i] = sum_j Wh[j, d] * P^T[j, i] on the PE in float32r
    with an appended ones column whose output row is the softmax
    denominator S,
  - scales P^T by 1/S (tensor_tensor against a partition-broadcast 1/S row,
    alternating VectorE / GpSimd) and stores per-head TRANSPOSED attention
    planes attnT_h[j, i] with fully contiguous DMAs.
The host transposes the per-head planes, stacks heads, and concatenates the
per-core row slices (pure memory reordering, no arithmetic).
"""

import numpy as np

N = 4096
D = 256
H = 4
OD = 64
NCORES = 8
IS = N // NCORES  # 512 rows of i per core
P = 128
NJC = N // P  # 32 j-chunks
NIC = IS // P  # 4 i-chunks per core

_cache = {}


def _build():
    from contextlib import ExitStack

    import concourse.bacc as bacc
    import concourse.mybir as mybir
    import concourse.tile as tile
    from concourse import masks

    dt = mybir.dt
    AF = mybir.ActivationFunctionType
    ALU = mybir.AluOpType

    nc = bacc.Bacc("TRN2", target_bir_lowering=False, debug=False)

    ht_d = nc.dram_tensor("hT", [D, N], dt.float32, kind="ExternalInput").ap()
    hit_d = nc.dram_tensor("hiT", [D, IS], dt.float32, kind="ExternalInput").ap()
    w_d = nc.dram_tensor("W", [D, D], dt.float32, kind="ExternalInput").ap()
    a_d = nc.dram_tensor("a", [H, 2 * OD], dt.float32, kind="ExternalInput").ap()

    attn_d = [
        nc.dram_tensor(f"attnT{hh}", [N, IS], dt.float32, kind="ExternalOutput").ap()
        for hh in range(H)
    ]
    outp_d = nc.dram_tensor("outp", [IS, D], dt.float32, kind="ExternalOutput").ap()

    with tile.TileContext(nc) as tc, ExitStack() as ctx:
        # ---------------- persistent small tensors ----------------
        const_pool = ctx.enter_context(tc.tile_pool(name="const", bufs=1))
        ident_f = const_pool.tile([P, P], dt.float32)
        masks.make_identity(nc, ident_f[:])
        ones_f = const_pool.tile([P, 1], dt.float32)
        nc.vector.memset(ones_f[:], 1.0)

        # AW[p, jc, h, 0:64] = Wh[jc*128+p, h*64:(h+1)*64]; AW[..., 64] = 1.0
        aw = const_pool.tile([P, NJC, H, OD + 1], dt.float32r)
        fj = const_pool.tile([P, NJC, H], dt.float32)
        fib = const_pool.tile([P, H, IS], dt.float32)
        sinv = const_pool.tile([P, NIC, H], dt.float32)
        out_sb = const_pool.tile([P, NIC, D], dt.float32)

        # ---------------- preamble (freed before the main loop) -------------
        with tc.tile_pool(name="pre", bufs=1) as pre, tc.tile_pool(
            name="pre_ps", bufs=2, space="PSUM"
        ) as pre_ps:
            hi_t = pre.tile([P, 2, IS], dt.float32)
            nc.sync.dma_start(hi_t[:], hit_d.rearrange("(dc p) n -> p dc n", p=P))
            w_sb = pre.tile([P, 2, D], dt.float32)
            nc.sync.dma_start(w_sb[:], w_d.rearrange("(k p) c -> p k c", p=P))
            h_t = pre.tile([P, 2, N], dt.float32)
            ht_re = ht_d.rearrange("(dc p) n -> p dc n", p=P)
            for kg in range(8):
                sl = slice(kg * 512, (kg + 1) * 512)
                nc.sync.dma_start(h_t[:, :, sl], ht_re[:, :, sl])

            # block-diagonal attention vectors: A_*[64*(hh%2):, hh//2, hh]
            a_dst = pre.tile([P, 2, H], dt.float32)
            a_src = pre.tile([P, 2, H], dt.float32)
            nc.vector.memset(a_dst[:], 0.0)
            nc.vector.memset(a_src[:], 0.0)
            for hh in range(H):
                ps = (hh % 2) * OD
                dc = hh // 2
                nc.sync.dma_start(
                    a_dst[ps : ps + OD, dc, hh : hh + 1],
                    a_d[hh : hh + 1, 0:OD].rearrange("o d -> d o"),
                )
                nc.sync.dma_start(
                    a_src[ps : ps + OD, dc, hh : hh + 1],
                    a_d[hh : hh + 1, OD : 2 * OD].rearrange("o d -> d o"),
                )

            # W^T (for v* = W @ a_*), then va/vd [d, h]
            w_t = pre.tile([P, 2, D], dt.float32)
            for cc in range(2):
                for dc in range(2):
                    tp = pre_ps.tile([P, P], dt.float32, tag="pre_tp", bufs=3)
                    nc.tensor.transpose(
                        tp[:], w_sb[:, dc, cc * P : (cc + 1) * P], ident_f[:]
                    )
                    nc.vector.tensor_copy(w_t[:, cc, dc * P : (dc + 1) * P], tp[:])

            # wva = [W | va] as the fused f32r moving operand
            wva = pre.tile([P, 2, D + H], dt.float32r)
            nc.vector.tensor_copy(wva[:, :, 0:D], w_sb[:])
            vd_sb = pre.tile([P, 2, H], dt.float32)
            for dc in range(2):
                ps_v = pre_ps.tile([P, H], dt.float32, tag="pre_sm")
                for cc in range(2):
                    nc.tensor.matmul(
                        ps_v[:],
                        w_t[:, cc, dc * P : (dc + 1) * P],
                        a_src[:, cc, :],
                        start=(cc == 0),
                        stop=(cc == 1),
                    )
                nc.vector.tensor_copy(wva[:, dc, D : D + H], ps_v[:])
                ps_v2 = pre_ps.tile([P, H], dt.float32, tag="pre_sm")
                for cc in range(2):
                    nc.tensor.matmul(
                        ps_v2[:],
                        w_t[:, cc, dc * P : (dc + 1) * P],
                        a_dst[:, cc, :],
                        start=(cc == 0),
                        stop=(cc == 1),
                    )
                nc.vector.tensor_copy(vd_sb[:, dc, :], ps_v2[:])

            # f_i rows for this core's slice -> partition-broadcast tiles
            for hh in range(H):
                ps_f = pre_ps.tile([1, IS], dt.float32, tag="pre_fi", bufs=1)
                for dc in range(2):
                    nc.tensor.matmul(
                        ps_f[:],
                        vd_sb[:, dc, hh : hh + 1],
                        hi_t[:, dc, :],
                        start=(dc == 0),
                        stop=(dc == 1),
                    )
                fi_row = pre.tile([1, IS], dt.float32, tag="fi_row")
                nc.vector.tensor_copy(fi_row[:], ps_f[:])
                nc.gpsimd.partition_broadcast(fib[:, hh, :], fi_row[:])

            # per h^T chunk: round to f32r, one fused matmul -> [Wh | f_j]
            h_rt = pre.tile([P, 2, N], dt.float32r)
            for kg in range(8):
                sl = slice(kg * 512, (kg + 1) * 512)
                nc.vector.tensor_copy(h_rt[:, :, sl], h_t[:, :, sl])
            for kt in range(NJC):
                ps_wh = pre_ps.tile([P, D + H], dt.float32, tag="pre_wh")
                for dc in range(2):
                    nc.tensor.matmul(
                        ps_wh[:],
                        h_rt[:, dc, kt * P : (kt + 1) * P],
                        wva[:, dc, :],
                        start=(dc == 0),
                        stop=(dc == 1),
                    )
                nc.vector.tensor_copy(
                    aw[:, kt, :, 0:OD],
                    ps_wh[:, 0:D].rearrange("p (hh o) -> p hh o", o=OD),
                )
                nc.vector.tensor_copy(fj[:, kt, :], ps_wh[:, D : D + H])

            nc.vector.tensor_copy(
                aw[:, :, :, OD : OD + 1],
                ones_f[:].to_broadcast([P, NJC, H, 1]),
            )

        # ---------------- main loop ----------------
        et_pool = ctx.enter_context(tc.tile_pool(name="et", bufs=3))
        pt_pool = ctx.enter_context(tc.tile_pool(name="pt", bufs=10))
        sc_pool = ctx.enter_context(tc.tile_pool(name="sc", bufs=4))
        agg_ps_pool = ctx.enter_context(
            tc.tile_pool(name="agg_ps", bufs=2, space="PSUM")
        )
        sm_ps_pool = ctx.enter_context(tc.tile_pool(name="sm_ps", bufs=2, space="PSUM"))
        sm_sb_pool = ctx.enter_context(tc.tile_pool(name="sm_sb", bufs=1))

        NJG = NJC // 4  # 8 groups of 4 j-chunks
        for hh in range(H):
            agg_ps = agg_ps_pool.tile([OD + 1, 512], dt.float32)
            pt_tiles = []
            for jg in range(NJG):
                et = et_pool.tile([P, 4, 512], dt.float32)
                for u in range(4):
                    jc = jg * 4 + u
                    nc.scalar.activation(
                        et[:, u, :],
                        fib[:, hh, :],
                        AF.Prelu,
                        bias=fj[:, jc, hh : hh + 1],
                        scale=1.0,
                        alpha=0.2,
                    )
                pt = pt_pool.tile(
                    [P, 4, 512], dt.float32r, name=f"pt{hh}_{jg}", tag="pt"
                )
                nc.scalar.activation(pt[:], et[:], AF.Exp)
                pt_tiles.append(pt)
                for u in range(4):
                    jc = jg * 4 + u
                    nc.tensor.matmul(
                        agg_ps[:],
                        aw[:, jc, hh, :],
                        pt[:, u, :],
                        start=(jc == 0),
                        stop=(jc == NJC - 1),
                    )

            # softmax denominators: S row -> broadcast -> 1/S (NR approx)
            s_row = sm_sb_pool.tile([1, 512], dt.float32, tag="s_row")
            nc.vector.tensor_copy(s_row[:], agg_ps[OD : OD + 1, :])
            s_b = sm_sb_pool.tile([P, 512], dt.float32, tag="s_b")
            nc.gpsimd.partition_broadcast(s_b[:], s_row[:])
            sinv_b = sm_sb_pool.tile([P, 512], dt.float32, tag="sinv_b")
            rec_tmp = sm_sb_pool.tile([P, 512], dt.float32, tag="rec_tmp")
            nc.vector.reciprocal_approx_accurate(sinv_b[:], s_b[:], rec_tmp[:])
            # 1/S columns for the out epilogue
            for c in range(NIC):
                ps_s = sm_ps_pool.tile([P, 1], dt.float32, tag="ps_s")
                nc.tensor.transpose(
                    ps_s[:], s_row[0:1, c * P : (c + 1) * P], ident_f[0:1, 0:1]
                )
                nc.vector.reciprocal(sinv[:, c, hh : hh + 1], ps_s[:])

            # scale P^T rows by 1/S[i] and store transposed attention planes
            for jg in range(NJG):
                sc = sc_pool.tile([P, 4, 512], dt.float32)
                eng = nc.gpsimd if (jg in (1, 4, 6) and hh < H - 1) else nc.vector
                eng.tensor_mul(
                    sc[:],
                    pt_tiles[jg][:],
                    sinv_b[:].unsqueeze(1).to_broadcast([P, 4, 512]),
                )
                nc.sync.dma_start(
                    attn_d[hh][jg * 512 : (jg + 1) * 512, :].rearrange(
                        "(u p) i -> p u i", p=P
                    ),
                    sc[:],
                )

            # aggregated output: transpose [65,512] psum back to i-major
            o_sb = sm_sb_pool.tile([OD + 1, 512], dt.float32, tag="o_sb")
            nc.vector.tensor_copy(o_sb[:], agg_ps[:])
            for c in range(NIC):
                ps_o = sm_ps_pool.tile([P, OD], dt.float32, tag="ps_o")
                nc.tensor.transpose(
                    ps_o[:], o_sb[0:OD, c * P : (c + 1) * P], ident_f[0:OD, 0:OD]
                )
                nc.vector.tensor_scalar_mul(
                    out_sb[:, c, hh * OD : (hh + 1) * OD],
                    ps_o[:],
                    sinv[:, c, hh : hh + 1],
                )

        nc.sync.dma_start(outp_d.rearrange("(c p) d -> p c d", p=P), out_sb[:])

    nc.compile()
    return nc


def kernel(h, W, a):
    from concourse.bass_utils import run_bass_kernel_spmd

    h = np.ascontiguousarray(np.asarray(h, dtype=np.float32))
    W = np.ascontiguousarray(np.asarray(W, dtype=np.float32))
    a = np.ascontiguousarray(np.asarray(a, dtype=np.float32))

    if "nc" not in _cache:
        _cache["nc"] = _build()
    nc = _cache["nc"]

    h_T = np.ascontiguousarray(h.T)
    in_maps = [
        {
            "hT": h_T,
            "hiT": np.ascontiguousarray(h_T[:, c * IS : (c + 1) * IS]),
            "W": W,
            "a": a,
        }
        for c in range(NCORES)
    ]
    res = run_bass_kernel_spmd(nc, in_maps, core_ids=list(range(NCORES)))
    _cache["last_results"] = res
    rs = res.results

    out = np.concatenate([r["outp"] for r in rs], axis=0)
    attn = np.empty((N, N, H), dtype=np.float32)
    for c, r in enumerate(rs):
        for hh in range(H):
            attn[c * IS : (c + 1) * IS, :, hh] = r[f"attnT{hh}"].T
    return out, attn
